# revision 3
# baseline (speedup 1.0000x reference)
"""Multi-head self-attention with RoPE on 8 Trainium2 NeuronCores — v2.

Full inputs in, full output out. Sharding: batch (2) x head-groups (4 heads
per core). Each core computes qkv projections for its heads, RoPE, full
softmax(QK^T)V, and a partial output projection; host sums the partials
per batch element and adds b_out.

v2 vs baseline:
  - all matmul operands bf16 (fast weight load, small DMA); fp32 PSUM accum
  - scores row-tiled: K=64 per head, two heads concurrent on the PE array
    (partition bases 0/64), writing halves of one [128,1024] PSUM tile so a
    single ACT exp instruction covers both heads
  - exp stays exact on ACT; projections / normalization / out-proj overlap
    under the ACT-bound attention phase via the tile scheduler
  - output partials written bf16

Problem shape: B=2, T=2048, D=1024, H=16, HD=64 (hardcoded).
"""

import os
os.environ.setdefault("NEURON_RT_RESET_CORES", "1")

import numpy as np
from contextlib import ExitStack

import concourse.bass as bass
import concourse.mybir as mybir
import concourse.tile as tile
from concourse import bass_utils

B, T, D, H = 2, 2048, 1024, 16
HD = 64          # head dim
HL = 4           # heads per core
N_CORES = 8
ROPE_BASE = 10000.0
NT = T // 128    # 16 key blocks
NK = D // 128    # 8 contraction chunks
NQ = T // 512    # 4 query blocks
SC = HD ** -0.5

F32 = mybir.dt.float32
F32R = mybir.dt.float32r
BF16 = mybir.dt.bfloat16

Exp = mybir.ActivationFunctionType.Exp

LAST_RESULTS = None
TRACE = False


def _split_excess_waits(nc, cap=1):
    """walrus in this env rejects >1 sync-wait per instruction; split extras
    onto single-wait NoOps on the same engine queue."""
    n = 0
    for f in nc.m.functions:
        for bb in f.blocks:
            insts = bb.instructions
            if not any(
                i.sync_info is not None and len(i.sync_info.on_wait) > cap
                for i in insts
            ):
                continue
            out = []
            for inst in insts:
                si = inst.sync_info
                waits = list(si.on_wait) if si is not None else []
                if len(waits) > cap:
                    extra, keep = waits[:-cap], waits[-cap:]
                    for k, w in enumerate(extra):
                        nop = mybir.InstNoOp(
                            name=f"{inst.name}-ws{k}",
                            engine=inst.engine,
                            sync_info=mybir.SyncInfo(on_wait=[w], on_update=[]),
                            bass_nofuse=True,
                        )
                        nc.register_instruction(nop)
                        out.append(nop)
                        n += 1
                    inst.sync_info = mybir.SyncInfo(
                        on_wait=keep, on_update=list(si.on_update)
                    )
                out.append(inst)
            bb.instructions = out
    return n


def _build_bass(with_qkv_bias, with_v_bias):
    nc = bass.Bass("TRN2", target_bir_lowering=False, debug=False, num_devices=1)

    # ---- DRAM I/O ----
    d_xT = nc.dram_tensor("xT", [D, T], BF16, kind="ExternalInput").ap()
    d_wqk = nc.dram_tensor("wqk", [D, 4 * 128], BF16, kind="ExternalInput").ap()
    d_wv = nc.dram_tensor("wv", [D, HL * (HD + 1)], BF16, kind="ExternalInput").ap()
    d_bqk = nc.dram_tensor("bqk", [1, 4 * 128], BF16, kind="ExternalInput").ap()
    d_bv = nc.dram_tensor("bv", [1, HL * (HD + 1)], BF16, kind="ExternalInput").ap()
    d_ones = nc.dram_tensor("ones", [1, 512], BF16, kind="ExternalInput").ap()
    d_cos = nc.dram_tensor("cos2", [64, T], F32, kind="ExternalInput").ap()
    d_sin = nc.dram_tensor("sin2", [64, T], F32, kind="ExternalInput").ap()
    d_rT = nc.dram_tensor("rT", [128, 128], BF16, kind="ExternalInput").ap()
    d_ind = nc.dram_tensor("ind", [2, 128], F32R, kind="ExternalInput").ap()
    d_amask = nc.dram_tensor("amask", [128, NT], F32, kind="ExternalInput").ap()
    d_wo = nc.dram_tensor("wo", [2 * 128, D], BF16, kind="ExternalInput").ap()
    d_out = nc.dram_tensor("out_part", [2, T, D], BF16, kind="ExternalOutput").ap()

    VW = HL * (HD + 1)  # 260

    with tile.TileContext(nc) as tc, ExitStack() as ctx:
        pool = lambda st, name, bufs: st.enter_context(tc.tile_pool(name=name, bufs=bufs))
        psum = lambda st, name, bufs: st.enter_context(
            tc.tile_pool(name=name, bufs=bufs, space="PSUM")
        )

        p_const = pool(ctx, "const", 1)
        p_xt = pool(ctx, "xt", NK)
        p_w = pool(ctx, "w", NK)
        p_wv = pool(ctx, "wv", NK)
        p_cs = pool(ctx, "cossin", 1)
        p_qk = pool(ctx, "qkT", 2)
        p_v = pool(ctx, "v", NT)
        p_u = pool(ctx, "u", 2)
        p_c = pool(ctx, "c", 2)
        p_e = pool(ctx, "eT", 4)
        p_at = pool(ctx, "at", 2)
        p_sum = pool(ctx, "sums", 1)
        p_s128 = pool(ctx, "s128", 2)
        p_an = pool(ctx, "an", 2)
        p_wo = pool(ctx, "wo", 2)
        p_osb = pool(ctx, "osb", 4)

        ps_s = psum(ctx, "ps_s", 2)    # [128,1024] : lead-in pqk/prot, S tiles, outproj1
        ps_pv = psum(ctx, "ps_pv", 1)  # [65,1024]  : PV accum
        ps_m = psum(ctx, "ps_m", 2)    # [128,512]x2: vproj, pair1 pqk/rot, norm, outproj0

        # ---- loads (ordered: small tables, then x by token-half, wo last) ----
        t_ones = p_const.tile([1, 512], BF16, tag="ones")
        nc.sync.dma_start(t_ones[:], d_ones[:])
        t_bqk = p_const.tile([1, 4 * 128], BF16, tag="bqk")
        nc.sync.dma_start(t_bqk[:], d_bqk[:])
        t_bv = p_const.tile([1, VW], BF16, tag="bv")
        nc.sync.dma_start(t_bv[:], d_bv[:])
        t_rT = p_const.tile([128, 128], BF16, tag="rT")
        nc.sync.dma_start(t_rT[:], d_rT[:])
        t_ind2 = p_const.tile([2, 128], F32R, tag="ind")
        nc.sync.dma_start(t_ind2[:], d_ind[:])
        t_amask = p_const.tile([128, NT], F32, tag="amask")
        nc.sync.dma_start(t_amask[:], d_amask[:])

        xt = [p_xt.tile([128, T], BF16, tag="xt", name=f"xt{k}") for k in range(NK)]
        wqk = []
        wv = []
        for k in range(NK):
            tw = p_w.tile([128, 4 * 128], BF16, tag="wqk")
            nc.sync.dma_start(tw[:], d_wqk[k * 128:(k + 1) * 128, :])
            wqk.append(tw)
        t_cos = p_cs.tile([128, T], F32, tag="cos")
        t_sin = p_cs.tile([128, T], F32, tag="sin")
        for hh in range(2):
            s = slice(hh * 1024, (hh + 1) * 1024)
            for k in range(NK):
                nc.sync.dma_start(xt[k][:, s], d_xT[k * 128:(k + 1) * 128, s])
            for half in range(2):
                p = slice(half * 64, (half + 1) * 64)
                nc.sync.dma_start(t_cos[p, s], d_cos[:, s])
                nc.sync.dma_start(t_sin[p, s], d_sin[:, s])
            if hh == 0:
                for k in range(NK):
                    tv = p_wv.tile([128, VW], BF16, tag="wv")
                    nc.sync.dma_start(tv[:], d_wv[k * 128:(k + 1) * 128, :])
                    wv.append(tv)
        wo_sb = []
        for c2 in range(2):
            wt = p_wo.tile([128, D], BF16, tag="wo")
            nc.sync.dma_start(wt[:], d_wo[c2 * 128:(c2 + 1) * 128, :])
            wo_sb.append(wt)

        # qT/kT per pair: [128, T] bf16, partitions 0:64 = head A, 64:128 = B
        qT = [p_qk.tile([128, T], BF16, tag="qT", name=f"qT{i}") for i in range(2)]
        kT = [p_qk.tile([128, T], BF16, tag="kT", name=f"kT{i}") for i in range(2)]

        # ---- qk projection + RoPE for one 128-feature block ----
        # blk: 0=q pair0, 1=k pair0, 2=q pair1, 3=k pair1
        def emit_qk_block(blk, dst, width, pool_ps, ihs=None):
            # width 1024 (lead-in, ps_s ring) or 512 (woven, ps_m)
            nchunk = T // width
            big = width == 1024
            for ih in (range(nchunk) if ihs is None else ihs):
                isl = slice(ih * width, (ih + 1) * width)
                pqk = pool_ps.tile([128, width], F32, tag=("s" if big else "m"), name="pqk")
                for k in range(NK):
                    for n5 in range(width // 512):
                        s5 = slice(n5 * 512, (n5 + 1) * 512)
                        g5 = slice(ih * width + n5 * 512, ih * width + (n5 + 1) * 512)
                        nc.tensor.matmul(
                            pqk[:, s5],
                            wqk[k][:, blk * 128:(blk + 1) * 128],
                            xt[k][:, g5],
                            start=(k == 0),
                            stop=(not with_qkv_bias and k == NK - 1),
                            skip_group_check=True,
                        )
                if with_qkv_bias:
                    for n5 in range(width // 512):
                        s5 = slice(n5 * 512, (n5 + 1) * 512)
                        nc.tensor.matmul(
                            pqk[:, s5],
                            t_bqk[:, blk * 128:(blk + 1) * 128],
                            t_ones[:, 0:512],
                            start=False, stop=True, skip_group_check=True,
                        )
                # RoPE: roped = raw*cos + R @ (raw*sin)  (sin is 32-periodic)
                u = p_u.tile([128, width], BF16, tag="u")
                nc.vector.tensor_mul(u[:], pqk[:], t_sin[:, isl])
                cc = p_c.tile([128, width], F32, tag="c")
                nc.vector.tensor_mul(cc[:], pqk[:], t_cos[:, isl])
                if big:
                    prot = pool_ps.tile([128, width], F32, tag="s", name="prot")
                    for n5 in range(2):
                        s5 = slice(n5 * 512, (n5 + 1) * 512)
                        nc.tensor.matmul(prot[:, s5], t_rT[:], u[:, s5],
                                         start=True, stop=True, skip_group_check=True)
                    nc.vector.tensor_add(dst[:, isl], cc[:], prot[:])
                else:
                    prot = ps_m.tile([128, 512], F32, tag="m", name="prot")
                    nc.tensor.matmul(prot[:], t_rT[:], u[:],
                                     start=True, stop=True, skip_group_check=True)
                    nc.vector.tensor_add(dst[:, isl], cc[:], prot[:])

        # ---- v projection for one token tile ----
        v_sb = [None] * NT

        def emit_vproj(t):
            pv_ps = ps_m.tile([128, 512], F32, tag="m", name="pv_ps")[:, 0:VW]
            for k in range(NK):
                nc.tensor.matmul(
                    pv_ps[:],
                    xt[k][:, t * 128:(t + 1) * 128],
                    wv[k][:],
                    start=(k == 0),
                    stop=(not with_v_bias and k == NK - 1),
                    skip_group_check=True,
                )
            vt = p_v.tile([128, VW], BF16, tag="v")
            if with_v_bias:
                nc.tensor.matmul(pv_ps[:], t_ones[:, 0:128], t_bv[:],
                                 start=False, stop=True, skip_group_check=True)
                nc.vector.tensor_copy(vt[:], pv_ps[:])
            else:
                nc.vector.tensor_copy(vt[:], pv_ps[:])
                ones_cols = vt[:].rearrange("p (h c) -> p h c", h=HL)[:, :, HD:HD + 1]
                nc.gpsimd.memset(ones_cols, 1.0)
            v_sb[t] = vt

        at = [None, None]
        sums_st = [None, None]
        s128_t = [None, None]
        r128_t = [None, None]
        recip2_t = [None, None]

        # ---- attention for one head pair ----
        an = [None, None]

        def emit_attention(pair, per_jb=None, post_qb=None):
            hA, hB = 2 * pair, 2 * pair + 1
            a = p_at.tile([128, T], F32, tag="at")
            ss = p_sum.tile([1, 2 * T], F32, tag="sums")
            s128 = p_s128.tile([128, 2 * NT], F32, tag="s128")
            r128 = p_s128.tile([128, 2 * NT], F32, tag="r128")
            recip2 = p_s128.tile([2, T], F32R, tag="recip2")
            anp = p_an.tile([128, T], BF16, tag="an")
            at[pair] = a
            an[pair] = anp
            s128_t[pair] = s128
            r128_t[pair] = r128
            recip2_t[pair] = recip2
            q, kk = qT[pair], kT[pair]
            for qb in range(NQ):
                qsl = slice(qb * 512, (qb + 1) * 512)
                pv = ps_pv.tile([65, 1024], F32, tag="pv")
                for jb in range(NT):
                    jsl = slice(jb * 128, (jb + 1) * 128)
                    S = ps_s.tile([128, 1024], F32, tag="s")
                    nc.tensor.matmul(S[:, 0:512], kk[0:64, jsl], q[0:64, qsl],
                                     start=True, stop=True, skip_group_check=True)
                    nc.tensor.matmul(S[:, 512:1024], kk[64:128, jsl], q[64:128, qsl],
                                     start=True, stop=True, skip_group_check=True)
                    E = p_e.tile([128, 1024], BF16, tag="e")
                    nc.scalar.activation(E[:], S[:], Exp,
                                         bias=t_amask[:, jb:jb + 1], scale=SC)
                    nc.tensor.matmul(
                        pv[:, 0:512],
                        v_sb[jb][:, hA * (HD + 1):(hA + 1) * (HD + 1)],
                        E[:, 0:512],
                        start=(jb == 0), stop=(jb == NT - 1),
                        skip_group_check=True,
                    )
                    nc.tensor.matmul(
                        pv[:, 512:1024],
                        v_sb[jb][:, hB * (HD + 1):(hB + 1) * (HD + 1)],
                        E[:, 512:1024],
                        start=(jb == 0), stop=(jb == NT - 1),
                        skip_group_check=True,
                    )
                    if per_jb is not None:
                        per_jb(qb, jb)
                # evacuate pv -> at / sums (DVE; head B shifts to partitions 64:128)
                nc.vector.tensor_copy(a[0:64, qsl], pv[0:64, 0:512])
                nc.vector.tensor_copy(a[64:128, qsl], pv[0:64, 512:1024])
                nc.vector.tensor_copy(ss[0:1, qb * 512:(qb + 1) * 512],
                                      pv[64:65, 0:512])
                nc.vector.tensor_copy(ss[0:1, T + qb * 512:T + (qb + 1) * 512],
                                      pv[64:65, 512:1024])
                for h01 in range(2):
                    nc.sync.dma_start(
                        s128[:, h01 * NT + qb * 4:h01 * NT + (qb + 1) * 4],
                        ss[0:1, h01 * T + qb * 512:h01 * T + (qb + 1) * 512]
                        .rearrange("o (p c) -> o p c", p=128),
                    )
                if post_qb is not None:
                    post_qb(qb)

        def emit_norm(pair):
            s128 = s128_t[pair]
            r128 = r128_t[pair]
            recip2 = recip2_t[pair]
            nc.vector.reciprocal(r128[:], s128[:])
            for h01 in range(2):
                for qb in range(NQ):
                    nc.sync.dma_start(
                        recip2[h01:h01 + 1, qb * 512:(qb + 1) * 512]
                        .rearrange("o (p c) -> o p c", p=128),
                        r128[:, h01 * NT + qb * 4:h01 * NT + (qb + 1) * 4].bitcast(F32R),
                    )
            a = an[pair]
            for qb in range(NQ):
                qsl = slice(qb * 512, (qb + 1) * 512)
                pb = ps_m.tile([128, 512], F32, tag="m", name="pb")
                nc.tensor.matmul(pb[:], t_ind2[:], recip2[:, qsl],
                                 start=True, stop=True, skip_group_check=True)
                nc.vector.tensor_mul(a[:, qsl], at[pair][:, qsl], pb[:])

        # ---- out projection for one pair ----
        def emit_outproj(pair, use_big_psum, ts=None):
            a = an[pair]
            for t in (range(NT) if ts is None else ts):
                tsl = slice(t * 128, (t + 1) * 128)
                if use_big_psum:
                    pp = ps_s.tile([128, 1024], F32, tag="s", name="pp")
                    for n5 in range(2):
                        s5 = slice(n5 * 512, (n5 + 1) * 512)
                        nc.tensor.matmul(pp[:, s5], a[:, tsl], wo_sb[pair][:, s5],
                                         start=True, stop=True, skip_group_check=True)
                    osb = p_osb.tile([128, D], BF16, tag="osb")
                    if t % 2 == 0:
                        nc.vector.tensor_copy(osb[:], pp[:])
                    else:
                        nc.scalar.copy(osb[:], pp[:])
                    nc.sync.dma_start(d_out[pair, tsl, :], osb[:])
                else:
                    for n5 in range(2):
                        s5 = slice(n5 * 512, (n5 + 1) * 512)
                        pp = ps_m.tile([128, 512], F32, tag="m", name="pp")
                        nc.tensor.matmul(pp[:], a[:, tsl], wo_sb[pair][:, s5],
                                         start=True, stop=True, skip_group_check=True)
                        osb = p_osb.tile([128, 512], BF16, tag="osb5")
                        nc.vector.tensor_copy(osb[:], pp[:])
                        nc.sync.dma_start(d_out[pair, tsl, s5], osb[:])

        # ================= emission order =================
        # lead-in: pair0 q/k proj, first token half first, + first v tiles
        emit_qk_block(0, qT[0], 1024, ps_s, ihs=[0])
        emit_qk_block(1, kT[0], 1024, ps_s, ihs=[0])
        emit_vproj(0)
        emit_vproj(1)
        emit_qk_block(0, qT[0], 1024, ps_s, ihs=[1])
        emit_qk_block(1, kT[0], 1024, ps_s, ihs=[1])
        emit_vproj(2)
        emit_vproj(3)

        def p0_per_jb(qb, jb):
            if qb == 0 and 4 + jb < NT:
                emit_vproj(4 + jb)

        emit_attention(0, per_jb=p0_per_jb)
        # pair1 proj (scheduler hoists into pair0-attention gaps)
        emit_qk_block(2, qT[1], 512, ps_m)
        emit_qk_block(3, kT[1], 512, ps_m)
        emit_norm(0)
        emit_attention(1)
        emit_outproj(0, use_big_psum=False)
        emit_norm(1)
        emit_outproj(1, use_big_psum=True)

    _split_excess_waits(nc)
    return nc


_NC_CACHE = {}


def _rope_tables():
    inv_freq = (1.0 / (ROPE_BASE ** (np.arange(0, HD, 2, dtype=np.float32) / HD))
                ).astype(np.float32)
    t = np.arange(T, dtype=np.float32)
    freqs = np.einsum("t,f->tf", t, inv_freq).astype(np.float32)  # (T, HD/2)
    emb = np.concatenate([freqs, freqs], axis=-1)                  # (T, HD)
    cosT = np.cos(emb).astype(np.float32).T                        # (HD, T)
    sinT = np.sin(emb).astype(np.float32).T
    cos2 = np.ascontiguousarray(np.tile(cosT, (2, 1)))             # (128, T)
    sin2 = np.ascontiguousarray(np.tile(sinT, (2, 1)))
    return cos2, sin2


def _rot_matrix():
    r = np.zeros((128, 128), dtype=np.float32)
    for p0 in (0, 64):
        for d in range(32):
            r[p0 + d, p0 + 32 + d] = -1.0
            r[p0 + 32 + d, p0 + d] = 1.0
    return np.ascontiguousarray(r.T)


def _bf16(a):
    import ml_dtypes
    return np.asarray(a, dtype=np.float32).astype(ml_dtypes.bfloat16)


def kernel(x, W_qkv, b_qkv, W_out, b_out, padding_mask):
    global _NC_CACHE, LAST_RESULTS
    x = np.asarray(x, dtype=np.float32)
    W_qkv = np.asarray(W_qkv, dtype=np.float32)
    b_qkv = np.asarray(b_qkv, dtype=np.float32)
    W_out = np.asarray(W_out, dtype=np.float32)
    b_out = np.asarray(b_out, dtype=np.float32)
    padding_mask = np.asarray(padding_mask)

    with_qkv_bias = bool(np.any(b_qkv[:2 * D]))
    with_v_bias = bool(np.any(b_qkv[2 * D:]))
    key = (with_qkv_bias, with_v_bias)
    if key not in _NC_CACHE:
        _NC_CACHE[key] = _build_bass(with_qkv_bias, with_v_bias)
    nc = _NC_CACHE[key]

    cos2, sin2 = _rope_tables()
    rT = _rot_matrix()

    ind = np.zeros((2, 128), dtype=np.float32)
    for f in range(128):
        ind[f // 64, f] = 1.0

    ones = np.ones((1, 512), dtype=np.float32)

    in_maps = []
    for c in range(N_CORES):
        b = c // 4
        g = c % 4
        q0 = g * HL * HD
        wq = W_qkv[:, q0:q0 + HL * HD]
        wk = W_qkv[:, D + q0:D + q0 + HL * HD]
        wv_flat = W_qkv[:, 2 * D + q0:2 * D + q0 + HL * HD]
        # wqk blocks: [q pair0 | k pair0 | q pair1 | k pair1]
        wqk = np.concatenate(
            [wq[:, 0:128], wk[:, 0:128], wq[:, 128:256], wk[:, 128:256]], axis=1
        )
        bq = b_qkv[q0:q0 + HL * HD]
        bk = b_qkv[D + q0:D + q0 + HL * HD]
        bqk = np.concatenate(
            [bq[0:128], bk[0:128], bq[128:256], bk[128:256]]
        ).reshape(1, -1)
        # v columns interleaved with a ones-slot column per head
        wv_aug = np.zeros((D, HL * (HD + 1)), dtype=np.float32)
        bv_aug = np.zeros((1, HL * (HD + 1)), dtype=np.float32)
        for h in range(HL):
            wv_aug[:, h * (HD + 1):h * (HD + 1) + HD] = wv_flat[:, h * HD:(h + 1) * HD]
            bv_aug[0, h * (HD + 1):h * (HD + 1) + HD] = \
                b_qkv[2 * D + q0 + h * HD:2 * D + q0 + (h + 1) * HD]
            bv_aug[0, h * (HD + 1) + HD] = 1.0
        amask = np.where(padding_mask[b], np.float32(-1e30), np.float32(0.0))
        amask = np.ascontiguousarray(amask.reshape(NT, 128).T.astype(np.float32))
        in_maps.append({
            "xT": _bf16(x[b].T),
            "wqk": _bf16(wqk),
            "wv": _bf16(wv_aug),
            "bqk": _bf16(bqk),
            "bv": _bf16(bv_aug),
            "ones": _bf16(ones),
            "cos2": np.ascontiguousarray(cos2[:64]),
            "sin2": np.ascontiguousarray(sin2[:64]),
            "rT": _bf16(rT),
            "ind": ind,
            "amask": amask,
            "wo": _bf16(np.ascontiguousarray(W_out[q0:q0 + HL * HD, :])),
        })

    res = bass_utils.run_bass_kernel_spmd(
        nc, in_maps, core_ids=list(range(N_CORES)), trace=TRACE,
    )
    LAST_RESULTS = res

    out = np.zeros((B, T, D), dtype=np.float64)
    for c in range(N_CORES):
        p = np.asarray(res.results[c]["out_part"]).astype(np.float64)
        out[c // 4] += p[0] + p[1]
    out += b_out.astype(np.float64)
    return out.astype(np.float32)
